# revision 9
# baseline (speedup 1.0000x reference)
"""Bass/Trainium2 kernel for nn_EntangleComplex.

The reference computes (x_real @ op, x_imag @ op) where op is a DIAGONAL
matrix with +-1 entries (elementwise product of diagonal CZ-style gates).
Hence x @ op == x * diag(op)[None, :] exactly (IEEE: off-diagonal terms
are exact zeros).  The device kernel is therefore a DMA-bound elementwise
multiply by a broadcast sign vector, data-parallel over the batch dim
across 8 NeuronCores with no communication.

Per core: 512 rows of x_real + 512 rows of x_imag (16 MiB in, 16 MiB
out).  The sign vector is DMA'd as one 16 KiB row and broadcast to all
128 SBUF partitions with K=1 PE matmuls against a ones vector, so DMA
traffic stays at the 32 MiB roofline.  Raw Bass (no Tile) with explicit
semaphores: loads on the SP HWDGE ring, stores on the Activation HWDGE
ring (so store waits never block load issue), multiplies on DVE.  This
avoids Tile's entry barrier and end-of-kernel drain/EVSEM butterfly.
"""

from contextlib import ExitStack

import numpy as np

import concourse.bacc as bacc
import concourse.mybir as mybir
from concourse.bass_utils import run_bass_kernel_spmd

N_CORES = 8
BATCH = 4096
DIM = 4096
ROWS = BATCH // N_CORES  # 512 rows of each of x_real/x_imag per core
P = 128                  # SBUF partition count
MM_N = 512               # PSUM bank free-dim limit per matmul
NT = 2 * ROWS // P       # 8 x-tiles of [128, DIM] per core
NJ = DIM // MM_N         # 8 broadcast chunks

_NC = None


def _build_program():
    global _NC
    if _NC is not None:
        return _NC
    nc = bacc.Bacc(enable_partition_id=False)
    dt = mybir.dt.float32
    xr = nc.declare_dram_parameter("xr", [ROWS, DIM], dt, isOutput=False)
    xi = nc.declare_dram_parameter("xi", [ROWS, DIM], dt, isOutput=False)
    d = nc.declare_dram_parameter("d", [1, DIM], dt, isOutput=False)
    yr = nc.declare_dram_parameter("yr", [ROWS, DIM], dt, isOutput=True)
    yi = nc.declare_dram_parameter("yi", [ROWS, DIM], dt, isOutput=True)

    def src_ap(i):
        t, ii = (xr, i) if i < NT // 2 else (xi, i - NT // 2)
        return t[ii * P:(ii + 1) * P, :]

    def dst_ap(i):
        t, ii = (yr, i) if i < NT // 2 else (yi, i - NT // 2)
        return t[ii * P:(ii + 1) * P, :]

    with ExitStack() as ctx:
        dsmall = ctx.enter_context(nc.sbuf_tensor("dsmall", [1, DIM], dt))
        ones = ctx.enter_context(nc.sbuf_tensor("ones", [1, P], dt))
        dtile = ctx.enter_context(nc.sbuf_tensor("dtile", [P, DIM], dt))
        xts = [
            ctx.enter_context(nc.sbuf_tensor(f"xt{i}", [P, DIM], dt))
            for i in range(NT)
        ]
        pbs = [
            ctx.enter_context(nc.psum_tensor(f"pb{j}", [P, MM_N], dt))
            for j in range(2)
        ]
        dsem = ctx.enter_context(nc.semaphore("dsem"))
        osem = ctx.enter_context(nc.semaphore("osem"))
        mmsem = ctx.enter_context(nc.semaphore("mmsem"))
        cpsem = ctx.enter_context(nc.semaphore("cpsem"))
        mulsem = ctx.enter_context(nc.semaphore("mulsem"))
        ssem = ctx.enter_context(nc.semaphore("ssem"))
        lsems = [ctx.enter_context(nc.semaphore(f"lsem{i}")) for i in range(NT)]
        block = ctx.enter_context(nc.Block())

        @block.sync
        def _(sync):
            sync.dma_start(dsmall[:], d[:]).then_inc(dsem, 16)
            for i in range(NT):
                sync.dma_start(xts[i][:], src_ap(i)).then_inc(lsems[i], 16)

        @block.tensor
        def _(tensor):
            tensor.wait_ge(osem, 1)
            tensor.wait_ge(dsem, 16)
            for j in range(NJ):
                if j >= 2:
                    # PSUM WAR: bank j%2 must have been copied out
                    tensor.wait_ge(cpsem, j - 1)
                nc.tensor.matmul(
                    pbs[j % 2][:],
                    ones[:],
                    dsmall[0:1, j * MM_N:(j + 1) * MM_N],
                    start=True,
                    stop=True,
                ).then_inc(mmsem, 1)

        @block.vector
        def _(vector):
            vector.memset(ones[:], 1.0).then_inc(osem, 1)
            for j in range(NJ):
                vector.wait_ge(mmsem, j + 1)
                vector.tensor_copy(
                    dtile[:, j * MM_N:(j + 1) * MM_N], pbs[j % 2][:]
                ).then_inc(cpsem, 1)
            # deep-pipeline RAW: muls read dtile, so wait for the copies'
            # writebacks even though they ran on this same engine
            vector.wait_ge(cpsem, NJ)
            for i in range(NT):
                vector.wait_ge(lsems[i], 16)
                vector.tensor_mul(xts[i][:], xts[i][:], dtile[:]).then_inc(
                    mulsem, 1
                )

        @block.scalar
        def _(scalar):
            for i in range(NT):
                scalar.wait_ge(mulsem, i + 1)
                scalar.dma_start(dst_ap(i), xts[i][:]).then_inc(ssem, 16)
            # outputs are in HBM once every store's sem receipt fired
            scalar.wait_ge(ssem, 16 * NT)

    nc.finalize()
    _NC = nc
    return nc


def kernel(x_real, x_imag, op):
    x_real = np.ascontiguousarray(np.asarray(x_real, dtype=np.float32))
    x_imag = np.ascontiguousarray(np.asarray(x_imag, dtype=np.float32))
    op = np.asarray(op, dtype=np.float32)
    dvec = np.ascontiguousarray(np.diagonal(op)).reshape(1, DIM)

    nc = _build_program()
    in_maps = []
    for c in range(N_CORES):
        sl = slice(c * ROWS, (c + 1) * ROWS)
        in_maps.append({"xr": x_real[sl], "xi": x_imag[sl], "d": dvec})
    res = run_bass_kernel_spmd(nc, in_maps, list(range(N_CORES))).results
    y_real = np.concatenate([r["yr"] for r in res], axis=0)
    y_imag = np.concatenate([r["yi"] for r in res], axis=0)
    return y_real, y_imag
